# revision 14
# baseline (speedup 1.0000x reference)
"""DigitCaps dynamic-routing kernel for 8 Trainium2 NeuronCores.

Strategy: shard the ROUTE dimension (4608 -> 576/core) instead of batch.
Per core, the x-shard and the packed W-shard fit in SBUF (bf16: 3.5MB), so
all 5 passes over u_hat (3x s-pass, 2x a-pass) recompute u_hat from
SBUF-resident data via PE matmuls -- no HBM re-streaming.  The only
cross-core traffic is one fused AllReduce per routing iteration carrying
s_unnorm [128,256] + the local softmax denominator [16].  The routing
logits b stay core-local (they are route-indexed).

v2 changes vs v1 (same math):
  - xpk is transposed on the HOST and DMA'd directly (the device-side
    DmaTransposeAnt cost 7.2us of SP-sequencer descriptor-gen and gated the
    first matmul to t=11.4us).  All inputs stream in 2-4 chunks spread over
    the sync/scalar/vector/gpsimd DMA queues so the it0 s-pass starts ~1.5us.
  - PE warm-up matmuls at t~0.3us and during each AllReduce window keep the
    tensor engine out of the low p-state (cold PE runs 0.65GHz vs 2.4GHz;
    ramp threshold is 3us of continuous busy).
  - sqrt(x) in the squash is computed as exp(0.5*ln(x)) so the ONLY
    activation table ever loaded is natural_log_exp (it also serves the
    prep exp(b)); v1 thrashed Exp<->Sqrt tables at 1.28us per reload.
  - a-pass wp*G multiply/reduce groups are split DVE:Pool 2:1 (Pool runs
    its own multiply+reduce stream), cutting the DVE-serialized a-pass
    from ~21us to ~14us per iteration.
  - the wc=W*exp(b) multiply is emitted in jc-quarters so the next s-pass
    can start on quarter 0 while later quarters are still computing.

Precision: matmul operands for the two big passes are bf16 (fp32 PSUM
accumulation); everything else fp32.
"""

import sys

if "/opt/trn_rl_repo" not in sys.path:
    sys.path.insert(0, "/opt/trn_rl_repo")

from contextlib import ExitStack

import ml_dtypes
import numpy as np

import concourse.bass as bass
import concourse.tile as tile
from concourse import bacc, bass_utils, mybir


def _pin_act_table():
    """Make the act-table chooser settle on natural_log_exp_and_others for
    every activation this kernel emits (Exp, Ln, Copy).  The chooser loads
    the FIRST json set containing the required func, so Exp picks
    exp_and_others and Ln picks natural_log -- thrashing 1.28us table loads
    on the squash critical path.  Hiding {exp, ln, copy} from every OTHER
    set is conservative: ids/order are unchanged and the set actually loaded
    really does contain every func used with it."""
    import functools

    from concourse import hw_specs

    orig = hw_specs.get_activation_tables
    if getattr(orig, "_digitcaps_pinned", False):
        return
    base = orig.__wrapped__ if hasattr(orig, "__wrapped__") else orig

    @functools.cache
    def patched(module_arch):
        tabs = dict(base(module_arch))
        tgt = "natural_log_exp_and_others"
        Exp = mybir.ActivationFunctionType.Exp
        Ln = mybir.ActivationFunctionType.Ln
        Copy = mybir.ActivationFunctionType.Copy
        mine = {Exp, Ln, Copy}
        if tgt not in tabs or not mine <= tabs[tgt]:
            return tabs  # unexpected table layout: leave untouched
        return {
            name: (funcs if name == tgt else funcs - mine)
            for name, funcs in tabs.items()
        }

    patched._digitcaps_pinned = True
    hw_specs.get_activation_tables = patched


_pin_act_table()

B, R, C, O, I = 128, 4608, 16, 16, 8
NCORES = 8
RL = R // NCORES          # 576 routes per core
RBLK = 16                 # routes per 128-partition block
NBLK = RL // RBLK         # 36
GBLK = 3                  # blocks per a-pass PSUM group
NG = NBLK // GBLK         # 12 groups
CO = C * O                # 256
JC = NBLK * C             # 576 (block, capsule) pairs
ITERS = 3
F32 = mybir.dt.float32
BF16 = mybir.dt.bfloat16


def _bc(ap, counts):
    """View `ap` broadcast along an appended 0-step free dim."""
    return bass.AP(tensor=ap.tensor, offset=ap.offset, ap=list(ap.ap) + [[0, counts]])


def _bc0(ap, counts):
    """View `ap` broadcast along a 0-step dim INSERTED before the free dims.
    Keeps the last dim packed so DVE 2x/4x modes stay eligible."""
    a = list(ap.ap)
    return bass.AP(tensor=ap.tensor, offset=ap.offset,
                   ap=[a[0], [0, counts]] + a[1:])


def _build(nc, reps=1, chain=False, ar_only=False, no_cc=False):
    f32 = F32
    xt_d = nc.dram_tensor("xt", [B, RL * I], BF16, kind="ExternalInput").ap()
    # host-transposed xpk[p=(rl,i), j, b] = x[b, r0 + 16j + (p>>3), p&7]
    xpk_d = nc.dram_tensor("xpk", [128, NBLK * B], BF16, kind="ExternalInput").ap()
    # host-packed, partition-major, o-MAJOR free: wp[p, (o, j, c)] =
    # W[r0+16j+(p>>3), c, o, p&7].  o-major keeps every DVE operand's last
    # dim packed (stride 1) so the wc multiply runs in 2x/4x mode.
    wp_d = nc.dram_tensor("wp", [128, O * NBLK * C], BF16, kind="ExternalInput").ap()
    ematT_d = nc.dram_tensor("ematT", [128, RBLK], BF16, kind="ExternalInput").ap()
    emat_d = nc.dram_tensor("emat", [RBLK, 128], BF16, kind="ExternalInput").ap()
    ones1_d = nc.dram_tensor("ones1", [1, 128], BF16, kind="ExternalInput").ap()
    ones16_d = nc.dram_tensor("ones16", [RBLK, 1], f32, kind="ExternalInput").ap()
    out_d = nc.dram_tensor("out", [B, C, O, 1], f32, kind="ExternalOutput").ap()

    NAR = B * CO + C  # AllReduce payload: s_unnorm then D (bf16 on the wire)
    cc_in = [nc.dram_tensor(f"cc_in{i}", [NAR], BF16) for i in range(ITERS * reps)]
    cc_out = [
        nc.dram_tensor(f"cc_out{i}", [NAR], BF16, addr_space="Shared")
        for i in range(ITERS * reps)
    ]
    groups = [list(range(NCORES))]

    with tile.TileContext(nc) as tc, ExitStack() as ctx:
        const = ctx.enter_context(tc.tile_pool(name="const", bufs=1))
        xpool = ctx.enter_context(tc.tile_pool(name="xpool", bufs=1))
        work = ctx.enter_context(tc.tile_pool(name="work", bufs=2))
        sq_pool = ctx.enter_context(tc.tile_pool(name="sq", bufs=2))
        ps_s = ctx.enter_context(tc.tile_pool(name="ps_s", bufs=1, space="PSUM"))
        ps_g = ctx.enter_context(tc.tile_pool(name="ps_g", bufs=2, space="PSUM"))
        ps_cex = ctx.enter_context(tc.tile_pool(name="ps_cex", bufs=1, space="PSUM"))
        ps_d = ctx.enter_context(tc.tile_pool(name="ps_d", bufs=1, space="PSUM"))

        dma = nc.sync.dma_start

        # --- PE warm-up: keep the tensor engine busy from ~0.3us so it is
        # fully ramped (2.4GHz needs >3us continuous busy) when the first
        # real s-matmul issues. warm tile has no input deps beyond memset.
        warm = const.tile([128, 512], BF16, tag="warm")
        nc.vector.memset(warm, 0.5)

        def warmup(n):
            warm_ps = ps_s.tile([16, 256], f32, tag="s")
            for _ in range(n):
                nc.tensor.matmul(warm_ps, lhsT=warm[:, 0:16], rhs=warm[:, 0:256],
                                 start=True, stop=True)

        warmup(9)

        # --- input streaming over the three DMA queues (SP/Activation/Pool),
        # first-needed first; subtile deps let the s-pass start on block 0
        # while later chunks are still in flight.
        xpk = xpool.tile([128, NBLK, B], BF16, tag="xpk")
        xpk3_d = xpk_d.rearrange("p (j b) -> p j b", b=B)
        wp_all = xpool.tile([128, O * NBLK * C], BF16, tag="wp")
        wp3 = wp_all.rearrange("p (o jc) -> p o jc", jc=JC)
        wp3_d = wp_d.rearrange("p (o jc) -> p o jc", jc=JC)
        xt_sb = xpool.tile([B, RL * I], BF16, tag="xt")
        H = JC // 2
        OH = O // 2
        Q = NBLK // 4

        # The two HWDGE queues carry everything the FIRST s-pass blocks need
        # (xpk q0 + both o-halves of wp's first 18 blocks); the slower SWDGE
        # (gpsimd) queue carries wp's back 18 blocks, needed ~2us later.
        nc.sync.dma_start(out=xpk[:, 0:Q, :], in_=xpk3_d[:, 0:Q, :])
        nc.sync.dma_start(out=wp3[:, OH:O, 0:H], in_=wp3_d[:, OH:O, 0:H])
        nc.sync.dma_start(out=xpk[:, Q:2 * Q, :], in_=xpk3_d[:, Q:2 * Q, :])
        nc.sync.dma_start(out=xt_sb[:, 0:RL * I // 2], in_=xt_d[:, 0:RL * I // 2])
        nc.sync.dma_start(out=xt_sb[:, RL * I // 2:], in_=xt_d[:, RL * I // 2:])
        nc.scalar.dma_start(out=wp3[:, 0:OH, 0:H], in_=wp3_d[:, 0:OH, 0:H])
        nc.scalar.dma_start(out=xpk[:, 2 * Q:3 * Q, :], in_=xpk3_d[:, 2 * Q:3 * Q, :])
        nc.scalar.dma_start(out=xpk[:, 3 * Q:NBLK, :], in_=xpk3_d[:, 3 * Q:NBLK, :])
        nc.gpsimd.dma_start(out=wp3[:, OH:O, H:JC], in_=wp3_d[:, OH:O, H:JC])
        nc.gpsimd.dma_start(out=wp3[:, 0:OH, H:JC], in_=wp3_d[:, 0:OH, H:JC])
        emat_sb = const.tile([RBLK, 128], BF16, tag="emat")
        nc.scalar.dma_start(out=emat_sb, in_=emat_d)
        ones16_sb = const.tile([RBLK, 1], f32, tag="ones16")
        nc.scalar.dma_start(out=ones16_sb, in_=ones16_d)
        ones1_sb = const.tile([1, 128], BF16, tag="ones1")
        nc.scalar.dma_start(out=ones1_sb, in_=ones1_d)
        ematT_sb = const.tile([128, RBLK], BF16, tag="ematT")
        nc.scalar.dma_start(out=ematT_sb, in_=ematT_d)
        dinit_sb = const.tile([1, C], BF16, tag="dinit")
        nc.vector.memset(dinit_sb, float(RL))

        # routing logits, core-local, split in two 18-block halves for
        # fine-grained pipelining: [16 (rl), 18, C] each
        b_halves = [
            const.tile([RBLK, NBLK // 2, C], f32, tag=f"b{h}", name=f"b{h}")
            for h in range(2)
        ]
        if chain:
            for bh in b_halves:
                nc.vector.memset(bh, 0.0)

        if ar_only:
            # isolate collective cost: 3*reps chained AllReduces, no compute
            seed = sq_pool.tile([B, CO], BF16, tag="s_sb")
            nc.vector.memset(seed, 1.0)
            dma(out=cc_in[0].ap()[0:B * CO].rearrange("(b n) -> b n", b=B), in_=seed)
            for k in range(ITERS * reps):
                nc.gpsimd.collective_compute(
                    "AllReduce", mybir.AluOpType.add, replica_groups=groups,
                    ins=[cc_in[k].ap()], outs=[cc_out[k].ap()],
                )
                if k + 1 < ITERS * reps:
                    t = sq_pool.tile([B, CO], BF16, tag="s_sb")
                    dma(out=t, in_=cc_out[k].ap()[0:B * CO].rearrange(
                        "(b n) -> b n", b=B))
                    dma(out=cc_in[k + 1].ap()[0:B * CO].rearrange(
                        "(b n) -> b n", b=B), in_=t)
            vtmp = sq_pool.tile([B, C, O], BF16, tag="vtmp")
            dma(out=vtmp,
                in_=cc_out[ITERS * reps - 1].ap()[0:B * CO].rearrange(
                    "(b c o) -> b c o", b=B, c=C))
            v_sb = sq_pool.tile([B, C, O], f32, tag="v")
            nc.vector.tensor_copy(out=v_sb, in_=vtmp)
        else:
            v_sb = None
            for rep in range(reps):
                v_sb = _routing(
                    nc, rep, wp_all, xpk, xt_sb, b_halves, emat_sb, ematT_sb,
                    ones1_sb, ones16_sb, dinit_sb, cc_in, cc_out, groups,
                    work, sq_pool, ps_s, ps_g, ps_cex, ps_cex, ps_d, dma,
                    warmup, chain=chain, no_cc=no_cc,
                )

        # --- output: v [128, 256] -> [128, 16, 16, 1] ---
        dma(out=out_d.rearrange("b c o a -> b (c o a)"), in_=v_sb)

    nc.compile()
    return nc


def _routing(
    nc, rep, wp_all, xpk, xt_sb, b_halves, emat_sb, ematT_sb,
    ones1_sb, ones16_sb, dinit_sb, cc_in, cc_out, groups,
    work, sq_pool, ps_s, ps_g, ps_cex, ps_m, ps_d, dma,
    warmup, chain=False, no_cc=False,
):
    f32 = F32
    NAR = B * CO + C
    HJ = NBLK // 2            # 18 blocks per half
    HJC = HJ * C              # 288
    HG = HJ // GBLK           # 6 a-pass groups per half
    # views of the o-major packed W: [p, o, (j,c)] and c-major [p, (j,c), o]
    wp_ojc = wp_all.rearrange("p (o jc) -> p o jc", jc=JC)      # [128, 16, 576]
    wp_cmaj = wp_all.rearrange("p (o jc) -> p jc o", jc=JC)     # [128, 576, 16]

    def prep_half(h):
        """exp(b) -> cex -> wc for blocks [h*18, h*18+18).  Separate tiles per
        half so Tile's dependency tracking lets the next s-pass half start
        while the other half is still in the a-pass.  wc is emitted in jc
        quarters so s-pass block 9h can start on quarter 0."""
        ebx = work.tile([RBLK, HJ, C], BF16, tag=f"ebx{h}")
        nc.scalar.activation(
            out=ebx, in_=b_halves[h], func=mybir.ActivationFunctionType.Exp
        )
        dpart = work.tile([RBLK, C], f32, tag=f"dpart{h}")
        nc.vector.reduce_sum(
            out=dpart, in_=ebx.rearrange("p n c -> p c n"),
            axis=mybir.AxisListType.X,
        )
        cex_ps = ps_cex.tile([128, HJC], f32, tag="cex")
        nc.tensor.matmul(cex_ps, lhsT=emat_sb,
                         rhs=ebx.rearrange("p n c -> p (n c)"),
                         start=True, stop=True)
        cex_sb = work.tile([128, HJC], BF16, tag=f"cex{h}")
        nc.vector.tensor_copy(out=cex_sb, in_=cex_ps)
        wc = work.tile([128, O, HJC], BF16, tag=f"wc{h}")
        WQ = HJC // 2
        for q in range(2):
            nc.vector.tensor_tensor(
                out=wc[:, :, q * WQ:(q + 1) * WQ],
                in0=wp_ojc[:, :, h * HJC + q * WQ:h * HJC + (q + 1) * WQ],
                in1=_bc0(cex_sb[:, q * WQ:(q + 1) * WQ], O),
                op=mybir.AluOpType.mult,
            )
        return wc.rearrange("p o jc -> p jc o"), dpart

    v_sb = None
    wc_halves = None
    dparts = None
    if chain:
        wc_halves, dparts = zip(prep_half(0), prep_half(1))
    for it in range(ITERS):
        weighted = chain or it > 0

        # --- s-pass: s_loc[b,(c,o)] = sum_j xpk_j.T @ rhs_j ---
        s_ps = ps_s.tile([B, CO], f32, tag="s")
        for j in range(NBLK):
            if weighted:
                rhs = wc_halves[j // HJ][:, (j % HJ) * C:((j % HJ) + 1) * C, :]
            else:
                rhs = wp_cmaj[:, j * C:(j + 1) * C, :]
            nc.tensor.matmul(
                s_ps, lhsT=xpk[:, j, :], rhs=rhs,
                start=(j == 0), stop=(j == NBLK - 1),
            )

        # --- fused AllReduce: [s_unnorm (32768) | D (16)] ---
        ci, co_ = cc_in[rep * ITERS + it].ap(), cc_out[rep * ITERS + it].ap()
        s_loc_sb = sq_pool.tile([B, CO], BF16, tag="s_loc")
        nc.scalar.activation(out=s_loc_sb, in_=s_ps,
                             func=mybir.ActivationFunctionType.Copy)
        nc.scalar.dma_start(out=ci[0:B * CO].rearrange("(b n) -> b n", b=B), in_=s_loc_sb)
        if weighted:
            dadd = work.tile([RBLK, C], f32, tag="dadd")
            nc.vector.tensor_add(out=dadd, in0=dparts[0], in1=dparts[1])
            dloc_ps = ps_d.tile([1, C], f32, tag="d")
            nc.tensor.matmul(dloc_ps, lhsT=ones16_sb, rhs=dadd,
                             start=True, stop=True)
            dloc_sb = work.tile([1, C], BF16, tag="dloc_sb")
            nc.vector.tensor_copy(out=dloc_sb, in_=dloc_ps)
            nc.scalar.dma_start(out=ci[B * CO:NAR].rearrange("(a c) -> a c", a=1), in_=dloc_sb)
        else:
            nc.scalar.dma_start(out=ci[B * CO:NAR].rearrange("(a c) -> a c", a=1), in_=dinit_sb)
        if no_cc:
            co_ = ci  # cost-model variant: skip the collective
        else:
            nc.gpsimd.collective_compute(
                "AllReduce", mybir.AluOpType.add, replica_groups=groups,
                ins=[ci], outs=[co_],
            )
        # keep PE out of the cold p-state through the AllReduce window
        warmup(6)
        s_sb = sq_pool.tile([B, CO], BF16, tag="s_sb")
        nc.scalar.dma_start(out=s_sb, in_=co_[0:B * CO].rearrange("(b n) -> b n", b=B))
        dg_sb = sq_pool.tile([1, C], BF16, tag="dg")
        nc.scalar.dma_start(out=dg_sb, in_=co_[B * CO:NAR].rearrange("(a c) -> a c", a=1))

        # --- squash with 1/D folded in:
        #   sn = (sum_o s_raw^2) / D^2;  fct = sn/((1+sn)sqrt(sn))
        #   v  = s_raw * fct / D
        # sqrt(sn) = exp(0.5*ln(sn)): Ln and Exp share one activation table
        # (natural_log_exp), so no LoadActFuncSet thrash with the prep Exp.
        dfull_ps = ps_d.tile([B, C], f32, tag="d")
        nc.tensor.matmul(dfull_ps, lhsT=ones1_sb, rhs=dg_sb, start=True, stop=True)
        dr = sq_pool.tile([B, C], f32, tag="dr")
        nc.vector.reciprocal(out=dr, in_=dfull_ps)
        sqs = sq_pool.tile([B, C, O], f32, tag="sqs")
        nc.vector.tensor_mul(
            out=sqs,
            in0=s_sb.rearrange("b (c o) -> b c o", o=O),
            in1=s_sb.rearrange("b (c o) -> b c o", o=O),
        )
        snr = sq_pool.tile([B, C], f32, tag="snr")
        nc.vector.reduce_sum(out=snr, in_=sqs, axis=mybir.AxisListType.X)
        dr2 = sq_pool.tile([B, C], f32, tag="dr2")
        nc.vector.tensor_mul(out=dr2, in0=dr, in1=dr)
        sn = sq_pool.tile([B, C], f32, tag="sn")
        nc.vector.tensor_mul(out=sn, in0=snr, in1=dr2)       # sn scaled
        lnsn = sq_pool.tile([B, C], f32, tag="lnsn")
        nc.scalar.activation(out=lnsn, in_=sn,
                             func=mybir.ActivationFunctionType.Ln)
        st = sq_pool.tile([B, C], f32, tag="st")
        nc.scalar.activation(out=st, in_=lnsn, scale=0.5,
                             func=mybir.ActivationFunctionType.Exp)
        w1 = sq_pool.tile([B, C], f32, tag="w1")
        nc.vector.tensor_mul(out=w1, in0=sn, in1=st)         # sn^1.5
        nc.vector.tensor_add(out=w1, in0=w1, in1=st)         # (1+sn)sqrt(sn)
        rc = sq_pool.tile([B, C], f32, tag="rc")
        nc.vector.reciprocal(out=rc, in_=w1)
        fct = sq_pool.tile([B, C], f32, tag="fct")
        nc.vector.tensor_mul(out=fct, in0=sn, in1=rc)        # sn/((1+sn)sqrt(sn))
        nc.vector.tensor_mul(out=fct, in0=fct, in1=dr)       # ... / D
        v_sb = sq_pool.tile([B, C, O], f32, tag="v")
        nc.vector.tensor_tensor(
            out=v_sb,
            in0=s_sb.rearrange("b (c o) -> b c o", o=O),
            in1=_bc(fct, O),
            op=mybir.AluOpType.mult,
        )

        # --- a-pass + next-iteration prep, pipelined per 18-block half.
        # Per wp*G group: Activation copies the G PSUM to bf16 SBUF so the
        # DVE multiply runs in 2x mode (every third group multiplies on Pool
        # straight from PSUM instead); the o-reduction + (1/B) i-contraction
        # then run on the PE as 16 per-o matmuls accumulating into the m
        # PSUM, so the DVE never touches a reduce.  The product is kept
        # o-major ([p, o, (j,c)]) so each per-o matmul rhs is contiguous;
        # v_bf is therefore copied o-major too. ---
        if it < ITERS - 1:
            v_bf = sq_pool.tile([B, O, C], BF16, tag="v_bf")
            nc.gpsimd.tensor_copy(out=v_bf,
                                  in_=v_sb.rearrange("b c o -> b o c"))
            wc_halves = [None, None]
            dparts = [None, None]
            for h in range(2):
                m_ps = ps_m.tile([RBLK, HJC], f32, tag="m")
                for g in range(HG):
                    g_ps = ps_g.tile([B, GBLK * CO], f32, tag="g")
                    for k in range(GBLK):
                        j = h * HJ + g * GBLK + k
                        nc.tensor.matmul(
                            g_ps[:, k * CO:(k + 1) * CO],
                            lhsT=xt_sb[:, j * 128:(j + 1) * 128],
                            rhs=v_bf.rearrange("b o c -> b (o c)"),
                            start=True, stop=True,
                        )
                    # pt[p, o, (j, c)] = wp * G, o-major
                    pt = work.tile([128, O, GBLK, C], BF16, tag="pt")
                    base = h * HJC + g * GBLK * C
                    wp_slice = wp_ojc[:, :, base:base + GBLK * C].rearrange(
                        "p o (j c) -> p o j c", c=C)
                    g_bf = work.tile([B, GBLK * CO], BF16, tag="g_bf")
                    nc.scalar.activation(
                        out=g_bf, in_=g_ps,
                        func=mybir.ActivationFunctionType.Copy)
                    mul_eng = nc.gpsimd if g % 3 == 2 else nc.vector
                    mul_eng.tensor_tensor(
                        out=pt, in0=wp_slice,
                        in1=g_bf.rearrange("p (j o c) -> p o j c", o=O, c=C),
                        op=mybir.AluOpType.mult,
                    )
                    # m[rl, (j,c)] += sum_i sum_o ematT[(rl,i)] * pt
                    pt2 = pt.rearrange("p o j c -> p o (j c)")
                    for o in range(O):
                        nc.tensor.matmul(
                            m_ps[:, g * GBLK * C:(g + 1) * GBLK * C],
                            lhsT=ematT_sb, rhs=pt2[:, o, :],
                            start=(o == 0), stop=(o == O - 1),
                        )
                bh = b_halves[h].rearrange("p n c -> p (n c)")
                if it == 0 and not chain:
                    nc.vector.tensor_copy(out=bh, in_=m_ps)
                else:
                    nc.vector.tensor_add(out=bh, in0=bh, in1=m_ps)
                # next-iteration weights for this half, immediately
                wc_halves[h], dparts[h] = prep_half(h)
    return v_sb


_NC_CACHE = None


def _get_nc():
    global _NC_CACHE
    if _NC_CACHE is None:
        nc = bacc.Bacc(
            "TRN2", target_bir_lowering=False, debug=False, num_devices=NCORES
        )
        _NC_CACHE = _build(nc)
    return _NC_CACHE


def make_in_maps(x, W):
    x = np.ascontiguousarray(np.asarray(x, np.float32))
    W = np.ascontiguousarray(np.asarray(W, np.float32))
    emat = np.zeros((RBLK, 128), np.float32)
    for rl in range(RBLK):
        emat[rl, rl * I:(rl + 1) * I] = 1.0
    ematT = (np.ascontiguousarray(emat.T) / np.float32(B)).astype(ml_dtypes.bfloat16)
    emat = emat.astype(ml_dtypes.bfloat16)
    ones1 = np.ones((1, 128), ml_dtypes.bfloat16)
    ones16 = np.ones((RBLK, 1), np.float32)
    in_maps = []
    for cid in range(NCORES):
        r0 = cid * RL
        xs = np.ascontiguousarray(x[:, r0:r0 + RL, :]).reshape(B, RL * I)
        xt = xs.astype(ml_dtypes.bfloat16)
        # xpk[p=(rl,i), j, b] = xt[b, 128j+p]
        xpk = (
            np.ascontiguousarray(xs.reshape(B, NBLK, 128).transpose(2, 1, 0))
            .reshape(128, NBLK * B)
            .astype(ml_dtypes.bfloat16)
        )
        Wl = W[0, r0:r0 + RL]  # [RL, C, O, I]
        # wp[p=(rl,i), (o, j, c)] -- o-major free layout
        wp = (
            np.ascontiguousarray(
                Wl.reshape(NBLK, RBLK, C, O, I).transpose(1, 4, 3, 0, 2)
            )
            .reshape(128, O * NBLK * C)
            .astype(ml_dtypes.bfloat16)
        )
        in_maps.append(
            {
                "xt": xt,
                "xpk": xpk,
                "wp": wp,
                "emat": emat,
                "ematT": ematT,
                "ones1": ones1,
                "ones16": ones16,
            }
        )
    return in_maps


def kernel(x, W):
    nc = _get_nc()
    in_maps = make_in_maps(x, W)
    res = bass_utils.run_bass_kernel_spmd(nc, in_maps, core_ids=list(range(NCORES)))
    out = np.asarray(res.results[0]["out"], np.float32)
    return out.reshape(B, C, O, 1)
